# revision 31
# baseline (speedup 1.0000x reference)
"""Multi-head attention (B=2, N=2048, C=1024, H=16, D=64) on 8 TRN2 NeuronCores.

Sharding: core c = (batch b = c//4) x (head-group g = c%4 -> heads 4g..4g+3).
Data parallel on B, tensor parallel on heads; fp16 ReduceScatter of the
out-projection partials within each 4-core batch group.

Everything on device stays transposed ([channel, position]); the host
pre-transposes inputs and post-transposes the output.

Pipeline: qk proj (DMA-paced, RoPE per n-chunk on DVE) -> v proj ->
attention chunk 0 -> attention chunk 1 with chunk-0's out-projection
matmuls interleaved as PE filler (keeps the PE p-state up and lets
chunk-0's ReduceScatter run under chunk-1's attention) -> chunk-1 proj
+ RS + readback (exposed tail).
"""

import numpy as np

import concourse.bacc as bacc
import concourse.tile as tile
import concourse.mybir as mybir
from concourse.bass_utils import run_bass_kernel_spmd

B, N, C, H = 2, 2048, 1024, 16
D = C // H          # 64
HL = H // 4         # 4 heads per core
CL = HL * D         # 256 local channels
N_CORES = 8
GROUPS = [[0, 1, 2, 3], [4, 5, 6, 7]]

F32 = mybir.dt.float32
BF16 = mybir.dt.float16
BF = np.float16

KC = C // 128       # 8  K-chunks of the input channel dim
NI = N // 512       # 4  512-wide i-chunks
NJ = N // 128       # 16 128-row j-chunks


def build_kernel(n_cores=N_CORES, groups=GROUPS):
    group_size = len(groups[0])
    rs_out_rows = C // group_size

    nc = bacc.Bacc("TRN2", target_bir_lowering=False, debug=False,
                   num_devices=n_cores)

    xT = nc.declare_dram_parameter("xT", [C, N], BF16, isOutput=False)
    cos2 = nc.declare_dram_parameter("cos2", [128, N], BF16, isOutput=False)
    sin2s = nc.declare_dram_parameter("sin2s", [128, N], BF16, isOutput=False)
    wqkT = nc.declare_dram_parameter("wqkT", [C, 2 * CL], BF16, isOutput=False)
    bqk = nc.declare_dram_parameter("bqk", [2 * CL, 1], F32, isOutput=False)
    wvT = nc.declare_dram_parameter("wvT", [C, CL], BF16, isOutput=False)
    wprojT = nc.declare_dram_parameter("wprojT", [CL, C], BF16, isOutput=False)
    # per-core bias, nonzero only on the rows this rank receives from the
    # ReduceScatter: adding it to the projection partials pre-RS makes the
    # RS output final (bias lands exactly once per row across the group)
    beffx = nc.declare_dram_parameter("beffx", [C, 1], F32, isOutput=False)
    # out is a flat sequence of contiguous [128, cw] ReduceScatter
    # destination blocks, one per (chunk, half), viewed as [1024, 512]
    out = nc.declare_dram_parameter("out", [1024, 512], BF16, isOutput=True)

    with tile.TileContext(nc) as tc:
        with tc.tile_pool(name="dram", bufs=1, space="DRAM") as dram, \
             tc.tile_pool(name="sbuf", bufs=1) as sb, \
             tc.tile_pool(name="psum", bufs=1, space="PSUM") as ps:

            # PSUM budget (8 banks of 2KB/partition):
            #   sc   [128,1024] f32 x2 bufs = 4 banks (qk accs / v pvs / scores)
            #   oacc [65,1024]  f32 x1 buf  = 2 banks (o accumulator)
            #   pp   [128,512]  f32 x2 bufs = 2 banks (proj partials / rb)

            # tile for clock-warming matmuls (see _warm_pe)
            warm = sb.tile([128, 128], BF16, name="warm", tag="warm")
            nc.vector.memset(warm[:], 0.001)

            def _warm_pe(tag, n):
                # short matmuls alternating two PSUM tiles: keeps the PE's
                # activity monitor busy so the clock gate stays at full rate
                wps = [ps.tile([128, 64], F32, name=f"warmp{tag}_{a}",
                               tag="pp", bufs=2) for a in range(2)]
                for r in range(n):
                    nc.tensor.matmul(wps[r % 2][:], warm[:], warm[:, :64],
                                     start=True, stop=True)

            # run a warm burst during the input-DMA dead window so the qk
            # projection starts with the clock gate already released
            _warm_pe("s", 24)

            # ---- load inputs ----
            # weights + rope tables first (small), then x in 512-column
            # blocks so the qk projection's n=0 iteration starts after ~1.5MB
            # instead of the full 5MB. Both HWDGE queues (sync + scalar)
            # share the load, split by kc parity.
            bqk_sb = []
            for m in range(4):
                t = sb.tile([128, 1], F32, name=f"bqk{m}", tag=f"bqk{m}")
                nc.sync.dma_start(t[:], bqk.ap()[128 * m:128 * (m + 1), :])
                bqk_sb.append(t)
            cos_sb = sb.tile([128, N], BF16, name="cos_sb", tag="cos_sb")
            nc.scalar.dma_start(cos_sb[:], cos2.ap())
            sin_sb = sb.tile([128, N], BF16, name="sin_sb", tag="sin_sb")
            nc.scalar.dma_start(sin_sb[:], sin2s.ap())
            wqk_sb = []
            for kc in range(KC):
                t = sb.tile([128, 2 * CL], BF16, name=f"wqk{kc}", tag=f"wqk{kc}")
                eng = nc.sync if kc % 2 == 0 else nc.scalar
                eng.dma_start(t[:], wqkT.ap()[128 * kc:128 * (kc + 1), :])
                wqk_sb.append(t)
            xb = [sb.tile([128, N], BF16, name=f"xb{kc}", tag=f"xb{kc}")
                  for kc in range(KC)]
            for n in range(NI):
                nsl = slice(512 * n, 512 * (n + 1))
                for kc in range(KC):
                    eng = nc.sync if kc % 2 == 0 else nc.scalar
                    eng.dma_start(xb[kc][:, nsl],
                                  xT.ap()[128 * kc:128 * (kc + 1), nsl])
            wv_sb = []
            for kc in range(KC):
                t = sb.tile([128, CL], BF16, name=f"wv{kc}", tag=f"wv{kc}")
                eng = nc.sync if kc % 2 == 0 else nc.scalar
                eng.dma_start(t[:], wvT.ap()[128 * kc:128 * (kc + 1), :])
                wv_sb.append(t)
            wproj_sb = []
            for p in range(2):
                t = sb.tile([128, C], BF16, name=f"wproj{p}", tag=f"wproj{p}")
                nc.sync.dma_start(t[:], wprojT.ap()[128 * p:128 * (p + 1), :])
                wproj_sb.append(t)
            beffx_sb = []
            for m in range(C // 128):
                t = sb.tile([128, 1], F32, name=f"beffx{m}", tag=f"beffx{m}")
                nc.scalar.dma_start(t[:], beffx.ap()[128 * m:128 * (m + 1), :])
                beffx_sb.append(t)

            # ---- qk projection + RoPE ----
            # chunk m rows: m=0:[q_h0,q_h1] m=1:[q_h2,q_h3] m=2:[k_h0,k_h1] m=3:[k_h2,k_h3]
            # so q and k of head h sit at the same partition offset 64*(h%2).
            # k of each head lands in its own zero-padded [128, N] tile so the
            # scores matmul can contract over K=128 (16-bit matmuls run at
            # half rate for K=64 -- zero rows buy back the full rate).
            q_r = [sb.tile([128, N], BF16, name=f"qkr{m}", tag=f"qkr{m}")
                   for m in range(2)]   # [q_h0,q_h1], [q_h2,q_h3]
            k_t = []      # 4 tiles: k_h at rows 64*(h%2), zeros elsewhere
            for h in range(4):
                kt = sb.tile([128, N], BF16, name=f"ktile{h}", tag=f"ktile{h}")
                z = slice(0, 64) if h % 2 == 1 else slice(64, 128)
                nc.vector.memset(kt[z, :], 0.0)
                k_t.append(kt)
            swap_mask = [i ^ 1 for i in range(32)]
            # kc-outer accumulation so the first matmul only needs the first
            # x/w chunk off DMA; RoPE runs per n-chunk on the DVE, overlapped
            # with the next n-chunk's matmuls instead of serially after.
            qks_t = [sb.tile([128, N], BF16, name=f"qks{m}", tag=f"qks{m}")
                     for m in range(4)]
            for n in range(NI):
                nsl = slice(512 * n, 512 * (n + 1))
                # acc slots alternate between PSUM tags so iteration n+1's
                # matmuls never wait on iteration n's evictions (which cost
                # ~3us of PE idle per iteration and reset the p-state ramp)
                if n % 2 == 0:
                    accs = [ps.tile([128, 1024], F32, name=f"qacc{n}_{a}",
                                    tag="sc", bufs=2) for a in range(2)]
                    aps = [accs[0][:, :512], accs[0][:, 512:],
                           accs[1][:, :512], accs[1][:, 512:]]
                else:
                    a01 = ps.tile([128, 1024], F32, name=f"qacc{n}_01",
                                  tag="oacc", bufs=1)
                    a2 = ps.tile([128, 512], F32, name=f"qacc{n}_2",
                                 tag="pp", bufs=2)
                    a3 = ps.tile([128, 512], F32, name=f"qacc{n}_3",
                                 tag="pp", bufs=2)
                    aps = [a01[:, :512], a01[:, 512:], a2[:], a3[:]]
                for kc in range(KC):
                    for m in range(4):
                        nc.tensor.matmul(
                            aps[m],
                            wqk_sb[kc][:, 128 * m:128 * (m + 1)],
                            xb[kc][:, nsl],
                            start=(kc == 0), stop=(kc == KC - 1))
                for m in range(4):
                    nc.scalar.activation(
                        qks_t[m][:, nsl], aps[m],
                        mybir.ActivationFunctionType.Identity,
                        bias=bqk_sb[m][:])
                for m in range(4):
                    qks = qks_t[m]
                    # RoPE: qk' = qks*cos2 + shift(qks)*sin2s
                    # (pair-swap of adjacent partitions via DVE stream shuffle)
                    shf = sb.tile([128, 512], BF16, name=f"shf{m}_{n}",
                                  tag="shf", bufs=2)
                    nc.vector.stream_shuffle(shf[:], qks[:, nsl], swap_mask)
                    t2 = sb.tile([128, 512], BF16, name=f"ropetmp{m}_{n}",
                                 tag="ropetmp", bufs=2)
                    nc.vector.tensor_mul(t2[:], shf[:], sin_sb[:, nsl])
                    if m < 2:
                        qkr = q_r[m]
                        nc.vector.tensor_mul(qkr[:, nsl], qks[:, nsl],
                                             cos_sb[:, nsl])
                        nc.vector.tensor_add(qkr[:, nsl], qkr[:, nsl], t2[:])
                    else:
                        t1 = sb.tile([128, 512], BF16, name=f"ropetc{m}_{n}",
                                     tag="ropetc", bufs=2)
                        nc.vector.tensor_mul(t1[:], qks[:, nsl],
                                             cos_sb[:, nsl])
                        h0, h1 = 2 * (m - 2), 2 * (m - 2) + 1
                        nc.vector.tensor_add(k_t[h0][0:64, nsl], t1[0:64, :],
                                             t2[0:64, :])
                        nc.vector.tensor_add(k_t[h1][64:128, nsl],
                                             t1[64:128, :], t2[64:128, :])

            # ---- v projection (natural [j, ch] layout, ones col appended per head) ----
            # j-chunks processed in pairs with the matmul stream alternating
            # between the two accumulators: back-to-back matmuls into the
            # same PSUM address serialize (~+330ns each), alternating ones
            # pipeline
            # 2 j-chunks accumulate into bank-separated halves of one PSUM
            # slot; groups alternate between the two sc slots so a group's
            # matmuls overlap the previous group's evictions (no PE stall).
            vaug = [None] * NJ
            for gq in range(NJ // 2):
                jcs = range(2 * gq, 2 * gq + 2)
                pv2 = ps.tile([128, 4 * CL], F32, name=f"pv2_{gq}",
                              tag="sc", bufs=2)
                for kc in range(KC):
                    for a, jc in enumerate(jcs):
                        nc.tensor.matmul(
                            pv2[:, 512 * a:512 * a + CL],
                            xb[kc][:, 128 * jc:128 * (jc + 1)],
                            wv_sb[kc][:],
                            start=(kc == 0), stop=(kc == KC - 1))
                for a, jc in enumerate(jcs):
                    va = sb.tile([128, HL * (D + 1)], BF16, name=f"vaug{jc}",
                                 tag=f"vaug{jc}")
                    nc.vector.memset(va[:, D::D + 1], 1.0)
                    nc.scalar.activation(
                        va.rearrange("p (h e) -> p h e", e=D + 1)[:, :, 0:D],
                        pv2[:, 512 * a:512 * a + CL].rearrange(
                            "p (h e) -> p h e", e=D)[:, :, :],
                        mybir.ActivationFunctionType.Copy)
                    vaug[jc] = va

            # per-partition bias AP used to shift scores before fp16 exp
            eshift = sb.tile([128, 1], F32, name="eshift", tag="eshift")
            nc.vector.memset(eshift[:], -16.0)
            # K=1 ones row used to broadcast denominators across partitions
            ones64 = sb.tile([1, 64], BF16, name="ones64", tag="ones64")
            nc.vector.memset(ones64[:], 1.0)

            # ---- attention + projection + RS, per i-chunk ----
            # trailing chunks are narrower so the final (exposed)
            # ReduceScatter halves cover only 512 columns
            chunks = [(0, 1024), (1024, 512), (1536, 512)]
            # flat row offset (in 512-wide out rows) of each RS block
            blk_rows = 128 * 1024 // 512      # rows per [128, 512]-equiv
            out_off = {}
            _off = 0
            for _ih, (_i0, _cw) in enumerate(chunks):
                for _half in range(2):
                    out_off[(_ih, _half)] = _off
                    _off += 128 * _cw // 512

            def finalize_head(ih, hl, oacc, o_pair, cw):
                # evict the o accumulator (numerator rows 0-63 to f32, row 64
                # = softmax denominator to fp16 for the K=1 broadcast matmul)
                # so the single PSUM oacc slot frees quickly, then normalize
                # from SBUF.
                den16 = sb.tile([1, cw], BF16, name=f"den{ih}_{hl}",
                                tag="den16", bufs=2)
                nc.vector.tensor_copy(den16[:], oacc[64:65, :])
                numS = sb.tile([64, cw], F32, name=f"numS{ih}_{hl}",
                               tag="numS", bufs=2)
                nc.vector.tensor_copy(numS[:], oacc[0:64, :])
                for q in range(cw // 512):
                    qsl = slice(512 * q, 512 * (q + 1))
                    rb = ps.tile([64, 512], F32, name=f"rb{ih}_{hl}_{q}",
                                 tag="pp", bufs=2)
                    nc.tensor.matmul(rb[:], ones64[:], den16[:, qsl],
                                     start=True, stop=True)
                    rr = sb.tile([64, 512], F32, name=f"rr{ih}_{hl}_{q}",
                                 tag="rr", bufs=2)
                    nc.vector.reciprocal_approx_fast(rr[:], rb[:])
                    nc.vector.tensor_mul(
                        o_pair[hl // 2][64 * (hl % 2):64 * (hl % 2) + 64, qsl],
                        numS[0:64, qsl], rr[:])

            # out-projection block emitters: each closure computes one
            # [128 out-ch, 512 col] partial, evicts to fp16 and DMAs it into
            # the chunk's RS input buffer. Interleaved into the NEXT chunk's
            # attention as PE filler (keeps the p-state up), or emitted
            # directly for the last chunk.
            def make_proj_blocks(ih, i0, cw, o_pair, rs_in):
                blocks = []
                ns = cw // 512
                for half in range(2):
                    for mc in range(4 * half, 4 * half + 4):
                        for n2 in range(ns):
                            def blk(mc=mc, n2=n2, ih=ih):
                                isl = slice(512 * n2, 512 * (n2 + 1))
                                pp = ps.tile([128, 512], F32,
                                             name=f"pp{ih}_{n2}_{mc}",
                                             tag="pp", bufs=2)
                                for p in range(2):
                                    nc.tensor.matmul(
                                        pp[:],
                                        wproj_sb[p][:, 128 * mc:128 * (mc + 1)],
                                        o_pair[p][:, isl],
                                        start=(p == 0), stop=(p == 1))
                                po = sb.tile([128, 512], BF16,
                                             name=f"po{ih}_{n2}_{mc}",
                                             tag="po", bufs=4)
                                # alternate evict engine so slots recycle 2x;
                                # bias folds in here (zero except owned rows)
                                if mc % 2 == 0:
                                    nc.vector.tensor_scalar_add(
                                        po[:], pp[:], beffx_sb[mc][:])
                                else:
                                    nc.scalar.activation(
                                        po[:], pp[:],
                                        mybir.ActivationFunctionType.Identity,
                                        bias=beffx_sb[mc][:])
                                nc.sync.dma_start(
                                    rs_in[128 * mc:128 * (mc + 1), isl], po[:])
                            blocks.append(blk)
                return blocks

            def emit_rs_half(ih, half, i0, cw, rs_in, last=False):
                # bias is already folded in, so the RS result is final: one
                # DRAM->DRAM fp16 copy into the output block, no compute
                # epilogue. (Collectives cannot write IO tensors directly.)
                rs_out_h = dram.tile([512 // group_size, cw], BF16,
                                     name=f"rsout{ih}_{half}",
                                     tag=f"rsout{ih}_{half}")
                nc.gpsimd.collective_compute(
                    "ReduceScatter", mybir.AluOpType.add,
                    replica_groups=groups,
                    ins=[rs_in[512 * half:512 * (half + 1), :]],
                    outs=[rs_out_h[:]])
                r0 = out_off[(ih, half)]
                nrows = 128 * cw // 512
                deng = nc.sync if last else nc.gpsimd
                deng.dma_start(
                    out.ap()[r0:r0 + nrows, :],
                    rs_out_h.rearrange("p (a q) -> (p a) q", q=512)[:, :])

            # pending proj work from the previous chunk, drip-fed into this
            # chunk's attention: (blocks, fire) where fire(k) is called with
            # the count of completed blocks to trigger RS halves.
            pending_proj = None

            for ih, (i0, cw) in enumerate(chunks):
                ns = cw // 512
                o_pair = [sb.tile([128, cw], BF16, name=f"opair{ih}_{p}",
                                  tag=f"opair{p}", bufs=2) for p in range(2)]
                if ih > 0 and pending_proj is None:
                    _warm_pe(f"c{ih}", 12)
                blocks, fire = pending_proj if pending_proj else ([], None)
                # place all filler blocks within the first 2 heads so the
                # previous chunk's RS halves fire as early as possible and
                # stream under heads 2-3
                nslots = 2 * NJ
                stride = max(1, nslots // len(blocks)) if blocks else 0
                nextb = 0

                def tick(slot):
                    nonlocal nextb
                    if blocks and slot % stride == stride - 1 \
                            and nextb < len(blocks):
                        blocks[nextb]()
                        nextb += 1
                        fire(nextb)

                for hl in range(4):
                    qT = q_r[hl // 2]
                    kT = k_t[hl]
                    oacc = ps.tile([65, cw], F32, name=f"oacc{ih}_{hl}",
                                   tag="oacc", bufs=1)
                    exs = []

                    def emit_o(jc, oacc=oacc, exs=exs, hl=hl):
                        for q in range(ns):
                            nc.tensor.matmul(
                                oacc[:, 512 * q:512 * (q + 1)],
                                vaug[jc][:, (D + 1) * hl:(D + 1) * (hl + 1)],
                                exs[jc][:, 512 * q:512 * (q + 1)],
                                start=(jc == 0), stop=(jc == NJ - 1))

                    for jc in range(NJ):
                        sc = ps.tile([128, cw], F32, name=f"sc{ih}_{hl}_{jc}",
                                     tag="sc", bufs=2)
                        for q in range(ns):
                            nc.tensor.matmul(
                                sc[:, 512 * q:512 * (q + 1)],
                                kT[:, 128 * jc:128 * (jc + 1)],
                                qT[:, i0 + 512 * q:i0 + 512 * (q + 1)],
                                start=True, stop=True)
                        ex = sb.tile([128, cw], BF16, name=f"ex{ih}_{hl}_{jc}",
                                     tag="ex", bufs=6)
                        # bias shifts all scores so fp16 exp can't overflow
                        # (softmax is shift-invariant, cancels in num/den)
                        nc.scalar.activation(ex[:], sc[:],
                                             mybir.ActivationFunctionType.Exp,
                                             scale=float(1.0 / np.sqrt(D)),
                                             bias=eshift[:])
                        exs.append(ex)
                        if jc >= 1:
                            emit_o(jc - 1)
                        tick(hl * NJ + jc)
                    emit_o(NJ - 1)
                    finalize_head(ih, hl, oacc, o_pair, cw)

                # drain any leftover filler blocks
                while blocks and nextb < len(blocks):
                    blocks[nextb]()
                    nextb += 1
                    fire(nextb)

                rs_in = dram.tile([C, cw], BF16, name=f"rsin{ih}",
                                  tag=f"rsin{ih}")
                my_blocks = make_proj_blocks(ih, i0, cw, o_pair, rs_in)
                fired = [False, False]

                last = ih == len(chunks) - 1

                def my_fire(k, ih=ih, i0=i0, cw=cw, rs_in=rs_in,
                            fired=fired, nb=len(my_blocks), last=last):
                    if k >= nb // 2 and not fired[0]:
                        fired[0] = True
                        emit_rs_half(ih, 0, i0, cw, rs_in, last)
                    if k >= nb and not fired[1]:
                        fired[1] = True
                        emit_rs_half(ih, 1, i0, cw, rs_in, last)

                if ih == len(chunks) - 1:
                    # last chunk: emit proj + RS + readback directly (exposed)
                    for k, b in enumerate(my_blocks):
                        b()
                        my_fire(k + 1)
                else:
                    pending_proj = (my_blocks, my_fire)

    nc.compile()
    return nc


def shard_inputs(x, rope, w_qkv, b_qkv, w_proj, b_proj,
                 n_cores=N_CORES, group_size=4):
    """Per-core input maps. Host-side transposes/casts are part of sharding."""
    rs_out_rows = C // group_size
    # fold the v-bias through the projection into an effective output bias
    b_v = b_qkv[2 * C:3 * C]
    b_eff = (b_proj + b_v @ w_proj.T).astype(np.float32)   # [C]

    in_maps = []
    for c in range(n_cores):
        b = (c // group_size) % B
        g = c % group_size
        heads = range(HL * g, HL * g + HL)

        xTb = np.ascontiguousarray(x[b].T).astype(BF)            # [C, N]

        cosT = rope[b].T[:D, :]                                   # [64, N]
        sinT = rope[b].T[D:, :]
        cos2 = np.vstack([cosT, cosT]).astype(BF)                 # [128, N]
        sgn = np.where(np.arange(128) % 2 == 0, -1.0, 1.0)[:, None]
        sin2s = (np.vstack([sinT, sinT]) * sgn).astype(BF)        # [128, N]

        # qk weight rows ordered [q_h0..q_h3, k_h0..k_h3]
        qk_rows = []
        bqk_rows = []
        for h in heads:
            qk_rows.append(w_qkv[D * h:D * (h + 1), :])           # q rows
            bqk_rows.append(b_qkv[D * h:D * (h + 1)])
        for h in heads:
            qk_rows.append(w_qkv[C + D * h:C + D * (h + 1), :])   # k rows
            bqk_rows.append(b_qkv[C + D * h:C + D * (h + 1)])
        wqk = np.vstack(qk_rows)                                  # [512, C]
        wqkT = np.ascontiguousarray(wqk.T).astype(BF)             # [C, 512]
        bqk_v = np.concatenate(bqk_rows).astype(np.float32)[:, None]

        h0 = HL * g
        wv = w_qkv[2 * C + D * h0:2 * C + D * h0 + CL, :]          # [256, C]
        wvT = np.ascontiguousarray(wv.T).astype(BF)                # [C, 256]

        wp = w_proj[:, D * h0:D * h0 + CL]                         # [C, 256]
        wprojT = np.ascontiguousarray(wp.T).astype(BF)             # [256, C]

        # each chunk's RS is split into two half-channel collectives, so
        # rank r receives channels [128r:128r+128) and [512+128r:512+128r+128)
        # (for group_size=4). beffx carries the bias only on the rows this
        # rank receives, so the RS-summed output gets it exactly once.
        r = c % group_size
        hr = 512 // group_size
        beffx = np.zeros((C, 1), dtype=np.float32)
        beffx[hr * r:hr * (r + 1), 0] = b_eff[hr * r:hr * (r + 1)]
        beffx[512 + hr * r:512 + hr * (r + 1), 0] = \
            b_eff[512 + hr * r:512 + hr * (r + 1)]

        in_maps.append({
            "xT": xTb, "cos2": cos2, "sin2s": sin2s,
            "wqkT": wqkT, "bqk": bqk_v, "wvT": wvT,
            "wprojT": wprojT, "beffx": beffx,
        })
    return in_maps


def assemble(results, n_cores=N_CORES, group_size=4):
    hr = 512 // group_size
    chunks = [(0, 1024), (1024, 512), (1536, 512)]
    out = np.empty((B, N, C), dtype=np.float32)
    for c in range(n_cores):
        b = (c // group_size) % B
        r = c % group_size
        flat = results[c]["out"].reshape(-1)   # fp16 RS blocks, flat
        off = 0
        for ih, (i0, cw) in enumerate(chunks):
            for half in range(2):
                blk = flat[off:off + 128 * cw].reshape(128, cw)
                off += 128 * cw
                out[b, i0:i0 + cw,
                    512 * half + hr * r:512 * half + hr * (r + 1)] = blk.T
    return out


_NC_CACHE = {}


def _get_nc():
    if "nc" not in _NC_CACHE:
        _NC_CACHE["nc"] = build_kernel()
    return _NC_CACHE["nc"]


def _run(inputs, trace=False, tmpdir=None):
    nc = _get_nc()
    inputs = {k: np.asarray(v) for k, v in inputs.items()}
    in_maps = shard_inputs(**inputs)
    res = run_bass_kernel_spmd(nc, in_maps, core_ids=list(range(N_CORES)),
                               trace=trace, tmpdir=tmpdir)
    return assemble(res.results), res


def kernel(**inputs):
    out, _ = _run(inputs)
    return out
